# revision 1
# baseline (speedup 1.0000x reference)
"""Cross-attention Trainium2 kernel (Bass/Tile), data-parallel over batch on 8 cores.

Reference computation per batch element b (no 1/sqrt(d) scaling):
    Q = S2[b] @ Wq            [N2, E]
    K = S1[b] @ Wk            [N1, E]
    V = S1[b] @ Wv            [N1, E]
    A = softmax(Q @ K^T, -1)  [N2, N1]
    out[b] = (A @ V) @ Wo + bo  [N2, D]

Device layout is fully transposed (feature dims on SBUF partitions):
    host supplies S1T = S1[b].T, S2T = S2[b].T  [D, N]
    phase A: KT[e, m], V[m, e] -> DRAM scratch (float32r)
    phase B per 512-query chunk:
      QT chunk -> scoresT tiles [m-part, n-free] -> exp (no max subtraction:
      |score| <= ~70 and exp stays in fp32 range) -> ones-matmul row sums
      accumulated in PSUM -> reciprocal -> gpsimd partition_broadcast ->
      UT = V.T @ E accumulated in PSUM (two 4-bank passes), normalized during
      PSUM eviction -> outT = Wo.T @ maskedT + bo -> DRAM [D, N2]; host
      transposes back.

All matmul operands are float32r (TF32-like 12-bit-mantissa rounding inside
the PE, full throughput at moving dim >= 256, ~1.6e-4 matmul rel err).
"""
import sys

sys.path.insert(0, "/opt/trn_rl_repo")

import numpy as np
from contextlib import ExitStack

P = 128
N_CORES = 8
B = 8          # batch (one element per core)
NQ = 2048      # queries (N2)
NK = 2048      # keys (N1)
D = 512        # query/cross dim
EI = 1024      # inner dim
CHUNK = 512    # query-chunk width (moving free dim)

_cache = {}


def _build(nq=NQ, nk=NK):
    import concourse.tile as tile
    from concourse import bacc, mybir

    F32 = mybir.dt.float32
    F32R = mybir.dt.float32r
    BF16 = mybir.dt.bfloat16
    Exp = mybir.ActivationFunctionType.Exp

    n_chunks = nq // CHUNK
    m_tiles = nk // P        # key tiles of 128
    e_tiles = EI // P        # 8
    d_tiles = D // P         # 4
    m_chunks = nk // CHUNK   # phase-A key chunks

    nc = bacc.Bacc("TRN2", target_bir_lowering=False, debug=False)

    S1T = nc.dram_tensor("S1T", [D, nk], F32R, kind="ExternalInput").ap()
    S2T = nc.dram_tensor("S2T", [D, nq], F32R, kind="ExternalInput").ap()
    Wq = nc.dram_tensor("Wq", [D, EI], F32R, kind="ExternalInput").ap()
    Wk = nc.dram_tensor("Wk", [D, EI], F32R, kind="ExternalInput").ap()
    Wv = nc.dram_tensor("Wv", [D, EI], F32R, kind="ExternalInput").ap()
    Wo = nc.dram_tensor("Wo", [EI, D], F32, kind="ExternalInput").ap()
    BO = nc.dram_tensor("BO", [P, d_tiles], F32, kind="ExternalInput").ap()
    OUT = nc.dram_tensor("OUT", [D, nq], F32, kind="ExternalOutput").ap()

    with tile.TileContext(nc) as tc, ExitStack() as ctx, \
            nc.allow_low_precision(reason="float32r staging for matmul operands"):
        const = ctx.enter_context(tc.tile_pool(name="const", bufs=1))
        w_pool = ctx.enter_context(tc.tile_pool(name="w_pool", bufs=1))
        dram = ctx.enter_context(tc.tile_pool(name="dram", bufs=1, space="DRAM"))
        ps_mm = ctx.enter_context(tc.tile_pool(name="ps_mm", bufs=3, space="PSUM"))
        ps_ut = ctx.enter_context(tc.tile_pool(name="ps_ut", bufs=4, space="PSUM"))
        ps_sum = ctx.enter_context(tc.tile_pool(name="ps_sum", bufs=1, space="PSUM"))

        # constants
        ones_f = const.tile([P, 1], F32, name="ones_f")
        nc.any.memset(ones_f[:], 1.0)
        ones_col = const.tile([P, 1], BF16, name="ones_col")
        nc.vector.tensor_copy(ones_col[:], ones_f[:])
        bo_t = const.tile([P, d_tiles], F32, name="bo_t")
        nc.sync.dma_start(bo_t[:], BO[:, :])

        # persistent weights: Wq as [p, d_tile, e], Wo as [p, e_tile, d]
        # (DMAs are emitted inside phase A, after the phase-A critical loads)
        wq_t = w_pool.tile([P, d_tiles, EI], F32R, name="wq_t")
        wo_t = w_pool.tile([P, e_tiles, D], BF16, name="wo_t")
        kt_res = w_pool.tile([P, e_tiles, nk], F32R, name="kt_res")

        # DRAM scratch for V (K^T stays SBUF-resident)
        v_d = dram.tile([m_tiles, P, EI], BF16, name="v_d")

        # ---------------- Phase A: KT and V ----------------
        with tc.tile_pool(name="pa_w", bufs=1) as pa_w, \
                tc.tile_pool(name="s1_pool", bufs=3) as s1_pool, \
                tc.tile_pool(name="evA", bufs=4) as evA, \
                nc.named_scope("phaseA"):
            wk_t = pa_w.tile([P, d_tiles, EI], F32R, name="wk_t")
            wk_r = Wk.rearrange("(t p) e -> p t e", p=P)
            wv_t = pa_w.tile([P, d_tiles, EI], F32R, name="wv_t")
            wv_r = Wv.rearrange("(t p) e -> p t e", p=P)

            s1_tiles = []
            s1_r = [
                S1T[:, mc * CHUNK:(mc + 1) * CHUNK].rearrange(
                    "(t p) m -> p t m", p=P)
                for mc in range(m_chunks)
            ]
            # chunk 0: interleave wk / s1 slices per d-tile so the first
            # accumulation group's operands arrive first
            s1_0 = s1_pool.tile([P, d_tiles, CHUNK], F32R, name="s1_t", tag="s1")
            s1_tiles.append(s1_0)
            for dt_ in range(d_tiles):
                nc.sync.dma_start(wk_t[:, dt_, :], wk_r[:, dt_, :])
                nc.sync.dma_start(s1_0[:, dt_, :], s1_r[0][:, dt_, :])
            for mc in range(1, m_chunks):
                s1_t = s1_pool.tile([P, d_tiles, CHUNK], F32R, name="s1_t", tag="s1")
                nc.sync.dma_start(s1_t[:], s1_r[mc])
                s1_tiles.append(s1_t)
                if mc == 1:
                    for dt_ in range(d_tiles):
                        nc.sync.dma_start(wv_t[:, dt_, :], wv_r[:, dt_, :])

            wq_r = Wq.rearrange("(t p) e -> p t e", p=P)
            for dt_ in range(d_tiles):
                nc.sync.dma_start(wq_t[:, dt_, :], wq_r[:, dt_, :])
            wo_r = Wo.rearrange("(t p) d -> p t d", p=P)
            for et_ in range(e_tiles):
                nc.gpsimd.dma_start(wo_t[:, et_, :], wo_r[:, et_, :])

            for mc in range(m_chunks):
                s1_t = s1_tiles[mc]
                # KT for m-chunk 0 first (unblocks chunk-0 scoresT), then V
                # before KT for later chunks (V feeds chunk-0 UT earlier)
                def _emit_kt(mc, s1_t):
                    for et in range(e_tiles):
                        acc = ps_mm.tile([P, CHUNK], F32, name="accA", tag="mm")
                        for dt_ in range(d_tiles):
                            nc.tensor.matmul(
                                acc[:],
                                wk_t[:, dt_, et * P:(et + 1) * P],
                                s1_t[:, dt_, :],
                                start=(dt_ == 0), stop=(dt_ == d_tiles - 1),
                            )
                        nc.vector.tensor_copy(
                            kt_res[:, et, mc * CHUNK:(mc + 1) * CHUNK], acc[:])

                def _emit_v(mc, s1_t):
                    for ml in range(CHUNK // P):
                        mt = mc * (CHUNK // P) + ml
                        for ec in range(EI // CHUNK):
                            accv = ps_mm.tile([P, CHUNK], F32, name="accV", tag="mm")
                            for dt_ in range(d_tiles):
                                nc.tensor.matmul(
                                    accv[:],
                                    s1_t[:, dt_, ml * P:(ml + 1) * P],
                                    wv_t[:, dt_, ec * CHUNK:(ec + 1) * CHUNK],
                                    start=(dt_ == 0), stop=(dt_ == d_tiles - 1),
                                )
                            evv = evA.tile([P, CHUNK], BF16, name="evV", tag="evV")
                            nc.vector.tensor_copy(evv[:], accv[:])
                            nc.sync.dma_start(
                                v_d[mt, :, ec * CHUNK:(ec + 1) * CHUNK], evv[:])

                if mc == 0:
                    _emit_kt(mc, s1_t)
                    _emit_v(mc, s1_t)
                else:
                    _emit_v(mc, s1_t)
                    _emit_kt(mc, s1_t)

        # ---------------- Phase B: attention ----------------
        s2_pool = ctx.enter_context(tc.tile_pool(name="s2_pool", bufs=2))
        qt_pool = ctx.enter_context(tc.tile_pool(name="qt_pool", bufs=2))
        e_pool = ctx.enter_context(tc.tile_pool(name="e_pool", bufs=m_tiles + 4))
        v_pool = ctx.enter_context(tc.tile_pool(name="v_pool", bufs=6))
        mk_pool = ctx.enter_context(tc.tile_pool(name="mk_pool", bufs=e_tiles + 2))
        out_pool = ctx.enter_context(tc.tile_pool(name="out_pool", bufs=2))
        misc = ctx.enter_context(tc.tile_pool(name="misc", bufs=2))

        for c in range(n_chunks):
          with nc.named_scope(f"chunk{c}"):
            csl = slice(c * CHUNK, (c + 1) * CHUNK)
            s2_t = s2_pool.tile([P, d_tiles, CHUNK], F32R, name="s2_t", tag="s2")
            nc.sync.dma_start(
                s2_t[:], S2T[:, csl].rearrange("(t p) n -> p t n", p=P))

            # QT chunk [e_tile, 128, CHUNK]
            qt_t = qt_pool.tile([P, e_tiles, CHUNK], F32R, name="qt_t", tag="qt")
            for et in range(e_tiles):
                accq = ps_mm.tile([P, CHUNK], F32, name="accQ", tag="mm")
                for dt_ in range(d_tiles):
                    nc.tensor.matmul(
                        accq[:],
                        wq_t[:, dt_, et * P:(et + 1) * P],
                        s2_t[:, dt_, :],
                        start=(dt_ == 0), stop=(dt_ == d_tiles - 1),
                    )
                nc.vector.tensor_copy(qt_t[:, et, :], accq[:])

            # scoresT tiles + exp + running column sums
            sum_ps = ps_sum.tile([1, CHUNK], F32, name="sum_ps", tag="sum")
            e_list = []
            for mt in range(m_tiles):
                acc_s = ps_mm.tile([P, CHUNK], F32, name="acc_s", tag="mm")
                for et in range(e_tiles):
                    nc.tensor.matmul(
                        acc_s[:],
                        kt_res[:, et, mt * P:(mt + 1) * P],
                        qt_t[:, et, :],
                        start=(et == 0), stop=(et == e_tiles - 1),
                    )
                e_t = e_pool.tile([P, CHUNK], BF16, name="e_t", tag="e")
                nc.scalar.activation(e_t[:], acc_s[:], Exp)
                e_list.append(e_t)
                nc.tensor.matmul(
                    sum_ps[:], ones_col[:], e_t[:],
                    start=(mt == 0), stop=(mt == m_tiles - 1),
                )

            # 1/sumexp broadcast to all partitions
            sum_sb = misc.tile([1, CHUNK], F32, name="sum_sb", tag="sumsb")
            nc.vector.tensor_copy(sum_sb[:], sum_ps[:])
            recip = misc.tile([1, CHUNK], F32, name="recip", tag="recip")
            nc.vector.reciprocal(recip[:], sum_sb[:])
            bc = misc.tile([P, CHUNK], F32, name="bc", tag="bc")
            nc.gpsimd.partition_broadcast(bc[:], recip[:])

            # UT = V^T @ E in two 4-bank passes; normalize on eviction
            masked = []
            for half in range(2):
                ut_list = [
                    ps_ut.tile([P, CHUNK], F32, name="ut", tag="ut")
                    for _ in range(4)
                ]
                for mt in range(m_tiles):
                    v_t = v_pool.tile([P, CHUNK], BF16, name="v_t", tag="v")
                    nc.sync.dma_start(
                        v_t[:], v_d[mt, :, half * CHUNK:(half + 1) * CHUNK])
                    for ei in range(4):
                        nc.tensor.matmul(
                            ut_list[ei][:],
                            v_t[:, ei * P:(ei + 1) * P],
                            e_list[mt][:],
                            start=(mt == 0), stop=(mt == m_tiles - 1),
                        )
                for ei in range(4):
                    m_t = mk_pool.tile([P, CHUNK], BF16, name="m_t", tag="mk")
                    nc.vector.tensor_mul(m_t[:], ut_list[ei][:], bc[:])
                    masked.append(m_t)

            # outT = Wo^T @ maskedT + bo
            for dt_ in range(d_tiles):
                acc_o = ps_mm.tile([P, CHUNK], F32, name="acc_o", tag="mm")
                for et in range(e_tiles):
                    nc.tensor.matmul(
                        acc_o[:],
                        wo_t[:, et, dt_ * P:(dt_ + 1) * P],
                        masked[et][:],
                        start=(et == 0), stop=(et == e_tiles - 1),
                    )
                o_sb = out_pool.tile([P, CHUNK], F32, name="o_sb", tag="osb")
                nc.vector.tensor_scalar_add(o_sb[:], acc_o[:], bo_t[:, dt_:dt_ + 1])
                nc.sync.dma_start(OUT[dt_ * P:(dt_ + 1) * P, csl], o_sb[:])

    nc.compile()
    return nc


def _get_nc(nq=NQ, nk=NK):
    key = (nq, nk)
    if key not in _cache:
        _cache[key] = _build(nq, nk)
    return _cache[key]


def kernel(S1, S2, Wq, Wk, Wv, Wo, bo, _trace=False):
    from concourse.bass_utils import run_bass_kernel_spmd

    S1 = np.asarray(S1, np.float32)
    S2 = np.asarray(S2, np.float32)
    b, nk, _ = S1.shape
    _, nq, _ = S2.shape
    nc = _get_nc(nq, nk)

    bo_r = np.ascontiguousarray(
        np.asarray(bo, np.float32).reshape(D // P, P).T)  # [128, d_tiles]
    wq = np.ascontiguousarray(np.asarray(Wq, np.float32))
    wk = np.ascontiguousarray(np.asarray(Wk, np.float32))
    wv = np.ascontiguousarray(np.asarray(Wv, np.float32))
    wo = np.ascontiguousarray(np.asarray(Wo, np.float32))

    in_maps = []
    for i in range(b):
        in_maps.append({
            "S1T": np.ascontiguousarray(S1[i].T),
            "S2T": np.ascontiguousarray(S2[i].T),
            "Wq": wq, "Wk": wk, "Wv": wv, "Wo": wo, "BO": bo_r,
        })

    res = run_bass_kernel_spmd(nc, in_maps, list(range(b)), trace=_trace)
    out = np.stack([np.asarray(res.results[i]["OUT"]).T for i in range(b)])
    if _trace:
        kernel.last_result = res
    return np.ascontiguousarray(out.astype(np.float32))



# revision 7
# speedup vs baseline: 2.4546x; 2.4546x over previous
"""Cross-attention Trainium2 kernel (Bass/Tile), data-parallel over batch on 8 cores.

Reference computation per batch element b (no 1/sqrt(d) scaling):
    Q = S2[b] @ Wq            [N2, E]
    K = S1[b] @ Wk            [N1, E]
    V = S1[b] @ Wv            [N1, E]
    A = softmax(Q @ K^T, -1)  [N2, N1]
    out[b] = (A @ V) @ Wo + bo  [N2, D]

Key algebraic reduction (inner dim E=1024 exceeds query dim D=512, so both
E-wide contractions collapse through associativity):
    scores = S2 (Wq Wk^T) S1^T          with M   = Wq @ Wk^T   [D, D] (host)
    out    = A (S1 (Wv Wo) + bo)        with WVO = Wv @ Wo     [D, D] (host)
bo folds into the value rows exactly because softmax rows sum to 1.
Per-core MACs drop from 12.9G to 5.4G; the output projection disappears.

Device layout (feature dims on SBUF partitions; host supplies transposes):
    phase A: VW[m, d] = S1^T-tiles^T @ WVO + bo  -> bf16, SBUF-resident
    phase B per 512-query chunk:
      TT[d, n] = M^T-tiles^T @ S2T chunk (f32r)
      scoresT tiles [m-part, n-free] = S1-tiles^T @ TT (f32r) -> exp to bf16
      (no max subtraction: |score| <= ~70, exp fits fp32/bf16 range)
      row sums via DVE partial-sum tree + ONE ones-matmul -> reciprocal ->
      gpsimd partition_broadcast; UT[d, n] accumulates VW^T @ E in PSUM,
      normalized by 1/sumexp during eviction -> DRAM [D, N2]; host transposes.
"""
import sys

sys.path.insert(0, "/opt/trn_rl_repo")

import numpy as np
from contextlib import ExitStack

P = 128
N_CORES = 8
B = 8          # batch (one element per core)
NQ = 2048      # queries (N2)
NK = 2048      # keys (N1)
D = 512        # query/cross dim
EI = 1024      # inner dim (eliminated on device by associativity)
CHUNK = 512    # query-chunk width (moving free dim)

_cache = {}


def _build(nq=NQ, nk=NK):
    import concourse.tile as tile
    from concourse import bacc, mybir

    F32 = mybir.dt.float32
    F32R = mybir.dt.float32r
    BF16 = mybir.dt.bfloat16
    Exp = mybir.ActivationFunctionType.Exp

    n_chunks = nq // CHUNK
    m_tiles = nk // P        # key tiles of 128
    d_tiles = D // P         # 4
    m_chunks = nk // CHUNK   # phase-A key chunks

    nc = bacc.Bacc("TRN2", target_bir_lowering=False, debug=False)

    S1T = nc.dram_tensor("S1T", [D, nk], F32R, kind="ExternalInput").ap()
    S2T = nc.dram_tensor("S2T", [D, nq], F32R, kind="ExternalInput").ap()
    M = nc.dram_tensor("M", [D, D], F32R, kind="ExternalInput").ap()
    WVO = nc.dram_tensor("WVO", [D, D], F32R, kind="ExternalInput").ap()
    BO = nc.dram_tensor("BO", [1, D], F32, kind="ExternalInput").ap()
    OUT = nc.dram_tensor("OUT", [D, nq], F32, kind="ExternalOutput").ap()

    with tile.TileContext(nc) as tc, ExitStack() as ctx, \
            nc.allow_low_precision(reason="f32r/bf16 staging for matmul operands"):
        const = ctx.enter_context(tc.tile_pool(name="const", bufs=1))
        w_pool = ctx.enter_context(tc.tile_pool(name="w_pool", bufs=1))
        ps_mm = ctx.enter_context(tc.tile_pool(name="ps_mm", bufs=3, space="PSUM"))
        ps_ut = ctx.enter_context(tc.tile_pool(name="ps_ut", bufs=4, space="PSUM"))
        ps_sum = ctx.enter_context(tc.tile_pool(name="ps_sum", bufs=1, space="PSUM"))

        # constants
        ones_f = const.tile([P, 1], F32, name="ones_f")
        nc.any.memset(ones_f[:], 1.0)
        ones_col = const.tile([P, 1], BF16, name="ones_col")
        nc.vector.tensor_copy(ones_col[:], ones_f[:])
        bo_row = const.tile([1, D], F32, name="bo_row")
        nc.sync.dma_start(bo_row[:], BO[:, :])
        bo_bc = const.tile([P, D], F32, name="bo_bc")
        nc.gpsimd.partition_broadcast(bo_bc[:], bo_row[:])

        # persistent tensors
        m_t = w_pool.tile([P, d_tiles, D], F32R, name="m_t")       # M[d', d]
        s1_res = w_pool.tile([P, d_tiles, nk], F32R, name="s1_res")  # S1T
        vw_t = w_pool.tile([P, m_tiles, D], BF16, name="vw_t")     # S1@WVO+bo

        # ---------------- Phase A: VW = S1 @ WVO + bo ----------------
        with tc.tile_pool(name="pa_w", bufs=1) as pa_w, \
                nc.named_scope("phaseA"):
            wvo_t = pa_w.tile([P, d_tiles, D], F32R, name="wvo_t")
            wvo_r = WVO.rearrange("(t p) d -> p t d", p=P)
            m_r = M.rearrange("(t p) d -> p t d", p=P)

            s1_r = [
                S1T[:, mc * CHUNK:(mc + 1) * CHUNK].rearrange(
                    "(t p) m -> p t m", p=P)
                for mc in range(m_chunks)
            ]
            # chunk 0: interleave wvo / s1 slices per d-tile so the first
            # accumulation group's operands arrive first
            for dt_ in range(d_tiles):
                nc.sync.dma_start(wvo_t[:, dt_, :], wvo_r[:, dt_, :])
                nc.sync.dma_start(
                    s1_res[:, dt_, 0:CHUNK], s1_r[0][:, dt_, :])
            for mc in range(1, m_chunks):
                nc.sync.dma_start(
                    s1_res[:, :, mc * CHUNK:(mc + 1) * CHUNK], s1_r[mc])
            for dt_ in range(d_tiles):
                nc.gpsimd.dma_start(m_t[:, dt_, :], m_r[:, dt_, :])

            for mc in range(m_chunks):
                for ml in range(CHUNK // P):
                    mt = mc * (CHUNK // P) + ml
                    accv = ps_mm.tile([P, D], F32, name="accV", tag="mm")
                    for dt_ in range(d_tiles):
                        nc.tensor.matmul(
                            accv[:],
                            s1_res[:, dt_, mt * P:(mt + 1) * P],
                            wvo_t[:, dt_, :],
                            start=(dt_ == 0), stop=(dt_ == d_tiles - 1),
                        )
                    nc.vector.tensor_add(vw_t[:, mt, :], accv[:], bo_bc[:])

        # ---------------- Phase B: attention ----------------
        s2_pool = ctx.enter_context(tc.tile_pool(name="s2_pool", bufs=2))
        tt_pool = ctx.enter_context(tc.tile_pool(name="tt_pool", bufs=2))
        e_pool = ctx.enter_context(tc.tile_pool(name="e_pool", bufs=m_tiles + 2))
        out_pool = ctx.enter_context(tc.tile_pool(name="out_pool", bufs=4))
        misc = ctx.enter_context(tc.tile_pool(name="misc", bufs=2))

        def emit_tt(c):
            """Load S2 chunk and compute TT[d, n] = M^T @ S2T chunk (f32r)."""
            csl = slice(c * CHUNK, (c + 1) * CHUNK)
            s2_t = s2_pool.tile([P, d_tiles, CHUNK], F32R, name="s2_t", tag="s2")
            nc.sync.dma_start(
                s2_t[:], S2T[:, csl].rearrange("(t p) n -> p t n", p=P))
            tt_t = tt_pool.tile([P, d_tiles, CHUNK], F32R, name="tt_t", tag="tt")
            for db in range(d_tiles):
                acct = ps_mm.tile([P, CHUNK], F32, name="accT", tag="mm")
                for dt_ in range(d_tiles):
                    nc.tensor.matmul(
                        acct[:],
                        m_t[:, dt_, db * P:(db + 1) * P],
                        s2_t[:, dt_, :],
                        start=(dt_ == 0), stop=(dt_ == d_tiles - 1),
                    )
                nc.vector.tensor_copy(tt_t[:, db, :], acct[:])
            return tt_t

        tt_cur = emit_tt(0)

        for c in range(n_chunks):
          with nc.named_scope(f"chunk{c}"):
            csl = slice(c * CHUNK, (c + 1) * CHUNK)
            tt_t = tt_cur

            # scoresT tiles + exp + DVE partial-sum tree over m-tiles
            sum_acc = misc.tile([P, CHUNK], BF16, name="sum_acc", tag="sacc")
            e_list = []
            for mt in range(m_tiles):
                acc_s = ps_mm.tile([P, CHUNK], F32, name="acc_s", tag="mm")
                for dt_ in range(d_tiles):
                    nc.tensor.matmul(
                        acc_s[:],
                        s1_res[:, dt_, mt * P:(mt + 1) * P],
                        tt_t[:, dt_, :],
                        start=(dt_ == 0), stop=(dt_ == d_tiles - 1),
                    )
                e_t = e_pool.tile([P, CHUNK], BF16, name="e_t", tag="e")
                nc.scalar.activation(e_t[:], acc_s[:], Exp)
                e_list.append(e_t)
                if mt == 0:
                    nc.vector.tensor_copy(sum_acc[:], e_t[:])
                else:
                    nc.vector.tensor_add(sum_acc[:], sum_acc[:], e_t[:])

            # prefetch next chunk's TT while the softmax chain runs on DVE
            if c + 1 < n_chunks:
                tt_cur = emit_tt(c + 1)

            # single ones-matmul contracts the 128 partitions of sum_acc
            sum_ps = ps_sum.tile([1, CHUNK], F32, name="sum_ps", tag="sum")
            nc.tensor.matmul(sum_ps[:], ones_col[:], sum_acc[:],
                             start=True, stop=True)

            # UT[d, n] = sum_mt VW^T @ E, normalized during eviction
            ut_list = [
                ps_ut.tile([P, CHUNK], F32, name="ut", tag="ut")
                for _ in range(d_tiles)
            ]
            for mt in range(m_tiles):
                for db in range(d_tiles):
                    nc.tensor.matmul(
                        ut_list[db][:],
                        vw_t[:, mt, db * P:(db + 1) * P],
                        e_list[mt][:],
                        start=(mt == 0), stop=(mt == m_tiles - 1),
                    )

            # 1/sumexp broadcast to all partitions
            sum_sb = misc.tile([1, CHUNK], F32, name="sum_sb", tag="sumsb")
            nc.vector.tensor_copy(sum_sb[:], sum_ps[:])
            recip = misc.tile([1, CHUNK], F32, name="recip", tag="recip")
            nc.vector.reciprocal(recip[:], sum_sb[:])
            bc = misc.tile([P, CHUNK], F32, name="bc", tag="bc")
            nc.gpsimd.partition_broadcast(bc[:], recip[:])

            for db in range(d_tiles):
                o_sb = out_pool.tile([P, CHUNK], F32, name="o_sb", tag="osb")
                nc.vector.tensor_mul(o_sb[:], ut_list[db][:], bc[:])
                nc.sync.dma_start(OUT[db * P:(db + 1) * P, csl], o_sb[:])

    nc.compile()
    return nc


def _get_nc(nq=NQ, nk=NK):
    key = (nq, nk)
    if key not in _cache:
        _cache[key] = _build(nq, nk)
    return _cache[key]


def kernel(S1, S2, Wq, Wk, Wv, Wo, bo, _trace=False):
    from concourse.bass_utils import run_bass_kernel_spmd

    S1 = np.asarray(S1, np.float32)
    S2 = np.asarray(S2, np.float32)
    b, nk, _ = S1.shape
    _, nq, _ = S2.shape
    nc = _get_nc(nq, nk)

    # host-side weight collapse (exact up to fp64 rounding)
    Wq = np.asarray(Wq, np.float64)
    Wk = np.asarray(Wk, np.float64)
    Wv = np.asarray(Wv, np.float64)
    Wo = np.asarray(Wo, np.float64)
    m = np.ascontiguousarray((Wq @ Wk.T).astype(np.float32))      # [D, D]
    wvo = np.ascontiguousarray((Wv @ Wo).astype(np.float32))      # [D, D]
    bo_r = np.ascontiguousarray(
        np.asarray(bo, np.float32).reshape(1, D))

    in_maps = []
    for i in range(b):
        in_maps.append({
            "S1T": np.ascontiguousarray(S1[i].T),
            "S2T": np.ascontiguousarray(S2[i].T),
            "M": m, "WVO": wvo, "BO": bo_r,
        })

    res = run_bass_kernel_spmd(nc, in_maps, list(range(b)), trace=_trace)
    out = np.stack([np.asarray(res.results[i]["OUT"]).T for i in range(b)])
    if _trace:
        kernel.last_result = res
    return np.ascontiguousarray(out.astype(np.float32))


# revision 11
# speedup vs baseline: 2.6485x; 1.0790x over previous
"""Cross-attention Trainium2 kernel (Bass/Tile), data-parallel over batch on 8 cores.

Reference computation per batch element b (no 1/sqrt(d) scaling):
    Q = S2[b] @ Wq            [N2, E]
    K = S1[b] @ Wk            [N1, E]
    V = S1[b] @ Wv            [N1, E]
    A = softmax(Q @ K^T, -1)  [N2, N1]
    out[b] = (A @ V) @ Wo + bo  [N2, D]

Key algebraic reduction (inner dim E=1024 exceeds query dim D=512, so both
E-wide contractions collapse through associativity):
    scores = S2 (Wq Wk^T) S1^T          with M   = Wq @ Wk^T   [D, D] (host)
    out    = A (S1 (Wv Wo) + bo)        with WVO = Wv @ Wo     [D, D] (host)
bo folds into the value rows exactly because softmax rows sum to 1.
Per-core MACs drop from 12.9G to 5.4G; the output projection disappears.

Device layout (feature dims on SBUF partitions; host supplies transposes):
    phase A: VW[m, d] = S1^T-tiles^T @ WVO + bo  -> bf16, SBUF-resident
    phase B per 512-query chunk:
      TT[d, n] = M^T-tiles^T @ S2T chunk (f32r)
      scoresT tiles [m-part, n-free] = S1-tiles^T @ TT (f32r) -> exp to bf16
      (no max subtraction: |score| <= ~70, exp fits fp32/bf16 range)
      row sums via DVE partial-sum tree + ONE ones-matmul -> reciprocal ->
      gpsimd partition_broadcast; UT[d, n] accumulates VW^T @ E in PSUM,
      normalized by 1/sumexp during eviction -> DRAM [D, N2]; host transposes.
"""
import sys

sys.path.insert(0, "/opt/trn_rl_repo")

import numpy as np
from contextlib import ExitStack

P = 128
N_CORES = 8
B = 8          # batch (one element per core)
NQ = 2048      # queries (N2)
NK = 2048      # keys (N1)
D = 512        # query/cross dim
EI = 1024      # inner dim (eliminated on device by associativity)
CHUNK = 512    # query-chunk width (moving free dim)

_cache = {}


def _build(nq=NQ, nk=NK):
    import concourse.tile as tile
    from concourse import bacc, mybir
    from concourse.bass_isa import ReduceOp

    F32 = mybir.dt.float32
    F32R = mybir.dt.float32r
    BF16 = mybir.dt.bfloat16
    Exp = mybir.ActivationFunctionType.Exp

    n_chunks = nq // CHUNK
    m_tiles = nk // P        # key tiles of 128
    d_tiles = D // P         # 4
    m_chunks = nk // CHUNK   # phase-A key chunks

    nc = bacc.Bacc("TRN2", target_bir_lowering=False, debug=False)

    S1T = nc.dram_tensor("S1T", [D, nk], F32R, kind="ExternalInput").ap()
    S2T = nc.dram_tensor("S2T", [D, nq], F32R, kind="ExternalInput").ap()
    M = nc.dram_tensor("M", [D, D], F32R, kind="ExternalInput").ap()
    WVO = nc.dram_tensor("WVO", [D, D], F32R, kind="ExternalInput").ap()
    BO = nc.dram_tensor("BO", [1, D], F32, kind="ExternalInput").ap()
    OUT = nc.dram_tensor("OUT", [D, nq], F32, kind="ExternalOutput").ap()

    with tile.TileContext(nc) as tc, ExitStack() as ctx, \
            nc.allow_low_precision(reason="f32r/bf16 staging for matmul operands"):
        const = ctx.enter_context(tc.tile_pool(name="const", bufs=1))
        w_pool = ctx.enter_context(tc.tile_pool(name="w_pool", bufs=1))
        ps_mm = ctx.enter_context(tc.tile_pool(name="ps_mm", bufs=4, space="PSUM"))
        ps_ut = ctx.enter_context(tc.tile_pool(name="ps_ut", bufs=4, space="PSUM"))

        # constants
        bo_row = const.tile([1, D], F32, name="bo_row")
        nc.sync.dma_start(bo_row[:], BO[:, :])
        bo_bc = const.tile([P, D], F32, name="bo_bc")
        nc.gpsimd.partition_broadcast(bo_bc[:], bo_row[:])

        # persistent tensors
        m_t = w_pool.tile([P, d_tiles, D], F32R, name="m_t")       # M[d', d]
        s1_res = w_pool.tile([P, d_tiles, nk], F32R, name="s1_res")  # S1T
        vw_t = w_pool.tile([P, m_tiles, D], BF16, name="vw_t")     # S1@WVO+bo

        # ---------------- Phase B pools (declared early: TT(0) precedes
        # phase-A VW in the PE stream to cover the S1/WVO load latency) ----
        s2_pool = ctx.enter_context(tc.tile_pool(name="s2_pool", bufs=2))
        tt_pool = ctx.enter_context(tc.tile_pool(name="tt_pool", bufs=2))
        e_pool = ctx.enter_context(tc.tile_pool(name="e_pool", bufs=m_tiles + 2))
        out_pool = ctx.enter_context(tc.tile_pool(name="out_pool", bufs=4))
        misc = ctx.enter_context(tc.tile_pool(name="misc", bufs=2))

        def emit_tt(c, s2_t=None):
            """Compute TT[d, n] = M^T @ S2T chunk (f32r)."""
            if s2_t is None:
                csl = slice(c * CHUNK, (c + 1) * CHUNK)
                s2_t = s2_pool.tile(
                    [P, d_tiles, CHUNK], F32R, name="s2_t", tag="s2")
                nc.sync.dma_start(
                    s2_t[:], S2T[:, csl].rearrange("(t p) n -> p t n", p=P))
            tt_t = tt_pool.tile([P, d_tiles, CHUNK], F32R, name="tt_t", tag="tt")
            for db in range(d_tiles):
                acct = ps_mm.tile([P, CHUNK], F32, name="accT", tag="mm")
                for dt_ in range(d_tiles):
                    nc.tensor.matmul(
                        acct[:],
                        m_t[:, dt_, db * P:(db + 1) * P],
                        s2_t[:, dt_, :],
                        start=(dt_ == 0), stop=(dt_ == d_tiles - 1),
                    )
                nc.vector.tensor_copy(tt_t[:, db, :], acct[:])
            return tt_t

        # ---------------- Phase A: TT(0), then VW = S1 @ WVO + bo --------
        with tc.tile_pool(name="pa_w", bufs=1) as pa_w, \
                nc.named_scope("phaseA"):
            wvo_t = pa_w.tile([P, d_tiles, D], F32R, name="wvo_t")
            wvo_r = WVO.rearrange("(t p) d -> p t d", p=P)
            m_r = M.rearrange("(t p) d -> p t d", p=P)

            # critical startup loads: S2(0) + M feed TT(0); M rides the
            # gpsimd DMA queue so it lands in parallel with the sync queue
            s2_0 = s2_pool.tile([P, d_tiles, CHUNK], F32R, name="s2_t", tag="s2")
            nc.sync.dma_start(
                s2_0[:], S2T[:, 0:CHUNK].rearrange("(t p) n -> p t n", p=P))
            for dt_ in range(d_tiles):
                nc.gpsimd.dma_start(m_t[:, dt_, :], m_r[:, dt_, :])

            s1_r = [
                S1T[:, mc * CHUNK:(mc + 1) * CHUNK].rearrange(
                    "(t p) m -> p t m", p=P)
                for mc in range(m_chunks)
            ]
            # interleave wvo / s1 slices per d-tile so the first VW
            # accumulation group's operands arrive first
            for dt_ in range(d_tiles):
                nc.sync.dma_start(wvo_t[:, dt_, :], wvo_r[:, dt_, :])
                nc.sync.dma_start(
                    s1_res[:, dt_, 0:CHUNK], s1_r[0][:, dt_, :])
            for mc in range(1, m_chunks):
                nc.sync.dma_start(
                    s1_res[:, :, mc * CHUNK:(mc + 1) * CHUNK], s1_r[mc])

            tt_cur = emit_tt(0, s2_t=s2_0)

            for mc in range(m_chunks):
                for ml in range(CHUNK // P):
                    mt = mc * (CHUNK // P) + ml
                    accv = ps_mm.tile([P, D], F32, name="accV", tag="mm")
                    for dt_ in range(d_tiles):
                        nc.tensor.matmul(
                            accv[:],
                            s1_res[:, dt_, mt * P:(mt + 1) * P],
                            wvo_t[:, dt_, :],
                            start=(dt_ == 0), stop=(dt_ == d_tiles - 1),
                        )
                    nc.vector.tensor_add(vw_t[:, mt, :], accv[:], bo_bc[:])

        for c in range(n_chunks):
          with nc.named_scope(f"chunk{c}"):
            csl = slice(c * CHUNK, (c + 1) * CHUNK)
            tt_t = tt_cur

            # scoresT tiles + exp + DVE partial-sum tree over m-tiles
            sum_acc = misc.tile([P, CHUNK], BF16, name="sum_acc", tag="sacc")
            e_list = []
            for mt in range(m_tiles):
                acc_s = ps_mm.tile([P, CHUNK], F32, name="acc_s", tag="mm")
                for dt_ in range(d_tiles):
                    nc.tensor.matmul(
                        acc_s[:],
                        s1_res[:, dt_, mt * P:(mt + 1) * P],
                        tt_t[:, dt_, :],
                        start=(dt_ == 0), stop=(dt_ == d_tiles - 1),
                    )
                e_t = e_pool.tile([P, CHUNK], BF16, name="e_t", tag="e")
                nc.scalar.activation(e_t[:], acc_s[:], Exp)
                e_list.append(e_t)
                if mt == 0:
                    nc.vector.tensor_copy(sum_acc[:], e_t[:])
                else:
                    nc.vector.tensor_add(sum_acc[:], sum_acc[:], e_t[:])

            # prefetch next chunk's TT while the softmax chain runs on DVE
            if c + 1 < n_chunks:
                tt_cur = emit_tt(c + 1)

            # gpsimd all-reduce contracts the 128 partitions of sum_acc and
            # broadcasts the result; reciprocal gives the softmax scale
            sums_bc = misc.tile([P, CHUNK], F32, name="sums_bc", tag="sbc")
            nc.gpsimd.partition_all_reduce(
                sums_bc[:], sum_acc[:], P, ReduceOp.add)
            bc = misc.tile([P, CHUNK], F32, name="bc", tag="bc")
            nc.vector.reciprocal(bc[:], sums_bc[:])

            # UT[d, n] = sum_mt VW^T @ E per d-block, normalized + stored
            # as soon as each block's accumulation completes
            for db in range(d_tiles):
                ut = ps_ut.tile([P, CHUNK], F32, name="ut", tag="ut")
                for mt in range(m_tiles):
                    nc.tensor.matmul(
                        ut[:],
                        vw_t[:, mt, db * P:(db + 1) * P],
                        e_list[mt][:],
                        start=(mt == 0), stop=(mt == m_tiles - 1),
                    )
                o_sb = out_pool.tile([P, CHUNK], F32, name="o_sb", tag="osb")
                nc.vector.tensor_mul(o_sb[:], ut[:], bc[:])
                nc.sync.dma_start(OUT[db * P:(db + 1) * P, csl], o_sb[:])

    nc.compile()
    return nc


def _get_nc(nq=NQ, nk=NK):
    key = (nq, nk)
    if key not in _cache:
        _cache[key] = _build(nq, nk)
    return _cache[key]


def kernel(S1, S2, Wq, Wk, Wv, Wo, bo, _trace=False):
    from concourse.bass_utils import run_bass_kernel_spmd

    S1 = np.asarray(S1, np.float32)
    S2 = np.asarray(S2, np.float32)
    b, nk, _ = S1.shape
    _, nq, _ = S2.shape
    nc = _get_nc(nq, nk)

    # host-side weight collapse (exact up to fp64 rounding)
    Wq = np.asarray(Wq, np.float64)
    Wk = np.asarray(Wk, np.float64)
    Wv = np.asarray(Wv, np.float64)
    Wo = np.asarray(Wo, np.float64)
    m = np.ascontiguousarray((Wq @ Wk.T).astype(np.float32))      # [D, D]
    wvo = np.ascontiguousarray((Wv @ Wo).astype(np.float32))      # [D, D]
    bo_r = np.ascontiguousarray(
        np.asarray(bo, np.float32).reshape(1, D))

    in_maps = []
    for i in range(b):
        in_maps.append({
            "S1T": np.ascontiguousarray(S1[i].T),
            "S2T": np.ascontiguousarray(S2[i].T),
            "M": m, "WVO": wvo, "BO": bo_r,
        })

    res = run_bass_kernel_spmd(nc, in_maps, list(range(b)), trace=_trace)
    out = np.stack([np.asarray(res.results[i]["OUT"]).T for i in range(b)])
    if _trace:
        kernel.last_result = res
    return np.ascontiguousarray(out.astype(np.float32))
